# revision 5
# baseline (speedup 1.0000x reference)
"""Trainium2 Bass kernel for ChannelDepsModule (sequential channel recurrence).

Math (per pixel, fp32):
    m_0 = mix_0 ; ybar_0 = round(x_0 - m_0) + m_0
    for i in 1..191:
        m_i = sum_{c<i} Wm[i-1,c] * ybar_c + b[i-1] + mix_i
        ybar_i = round(x_i - m_i) + m_i
    outputs: ybar, mix_out (= m)

Device strategy (per core, one batch image, 4096 pixels):
  - pixels on SBUF partitions ([128] x 32 chunks), channels on the free dim
  - channels processed in 6 blocks of 32:
      * cross-block contributions P via TensorE matmuls
        (lhsT = decoded ybar in channel-partition layout, rhs = Wm^T slice)
        accumulated in PSUM with pixel-partition output
      * in-block recurrence on VectorE: per channel a broadcast-multiply +
        grouped reduce for the dot, then round via the +-1.5*2^23 magic
        constant (IEEE RNE == jnp.round), assembled with scalar_tensor_tensor
      * each finished block is TensorE-transposed into the channel-partition
        ysb tiles used by later blocks' matmuls
  - b is folded into mix on the host; mix_out channel 0 is restored on host
"""

import os
import sys

import numpy as np

if "/opt/trn_rl_repo" not in sys.path:
    sys.path.insert(0, "/opt/trn_rl_repo")

N, C, H, Wd = 8, 192, 64, 64
NPIX = H * Wd          # 4096 pixels per core
B = 32                 # channel block size
NBLK = C // B          # 6
ROUND_C = 1.5 * 2.0**23  # fp32 add of this rounds to nearest-even integer

_CACHE = {}


def _build(n_chunks):
    """Build + compile the per-core Bass module. n_chunks pixel chunks of 128."""
    import concourse.bacc as bacc
    import concourse.mybir as mybir
    from concourse.tile import TileContext

    npix = n_chunks * 128
    fp32 = mybir.dt.float32
    Alu = mybir.AluOpType

    nc = bacc.Bacc(None, target_bir_lowering=False)

    xt = nc.dram_tensor("xt", [npix, C], fp32, kind="ExternalInput")
    mixt = nc.dram_tensor("mixt", [npix, C], fp32, kind="ExternalInput")
    wt = nc.dram_tensor("wt", [C, C], fp32, kind="ExternalInput")
    wtri = nc.dram_tensor("wtri", [1, NBLK * B * B], fp32, kind="ExternalInput")
    ident = nc.dram_tensor("ident", [128, 128], fp32, kind="ExternalInput")
    yt = nc.dram_tensor("yt", [npix, C], fp32, kind="ExternalOutput")
    mot = nc.dram_tensor("mot", [npix, C], fp32, kind="ExternalOutput")

    K = n_chunks  # pixel chunks

    with TileContext(nc) as tc:
        with (
            tc.tile_pool(name="big", bufs=1) as big,
            tc.tile_pool(name="small", bufs=1) as small,
            tc.tile_pool(name="scratch", bufs=2) as scratch,
            tc.tile_pool(name="psum", bufs=2, space="PSUM") as psum,
            tc.tile_pool(name="psumt", bufs=2, space="PSUM") as psumt,
        ):
            # Big pixel-partition tiles, free layout = k*192 + c
            X = big.tile([128, K * C], fp32, tag="X")
            MIX = big.tile([128, K * C], fp32, tag="MIX")  # becomes mix_out
            XMB = big.tile([128, K * C], fp32, tag="XMB")
            Y = big.tile([128, K * C], fp32, tag="Y")
            # channel-partition decoded ybar (for matmuls): chans 0-127 / 128-159
            ysb_lo = big.tile([128, npix], fp32, tag="ysb_lo")
            ysb_hi = big.tile([64, npix], fp32, tag="ysb_hi")

            wt_lo = small.tile([128, C], fp32, tag="wt_lo")
            wt_hi = small.tile([64, C], fp32, tag="wt_hi")
            wtri_t = small.tile([1, NBLK * B * B], fp32, tag="wtri")
            wtri_b = small.tile([128, NBLK * B * B], fp32, tag="wtri_b")
            id_t = small.tile([128, 128], fp32, tag="ident")

            def big_in(tile, dram):
                nc.sync.dma_start(
                    tile[:].rearrange("p (k c) -> p k c", c=C),
                    dram[:].rearrange("(k p) c -> p k c", p=128),
                )

            big_in(X, xt)
            big_in(MIX, mixt)
            nc.sync.dma_start(wt_lo[:], wt[0:128, :])
            nc.sync.dma_start(wt_hi[:], wt[128:C, :])
            nc.sync.dma_start(wtri_t[:], wtri[:])
            nc.sync.dma_start(id_t[:], ident[:])
            nc.gpsimd.partition_broadcast(wtri_b[:], wtri_t[:])

            # XMB = X - (MIX + b)  (b folded into MIX on host)
            nc.vector.tensor_sub(XMB[:], X[:], MIX[:])

            # strided [128, 32] view over chunk axis for channel ch
            def col(tile, ch):
                return tile[:].rearrange("p (k c) -> p k c", c=C)[:, :, ch]

            for sb in range(NBLK):
                base = sb * B

                # ---- P phase: cross-block contributions for this block ----
                if sb > 0:
                    PP = psum.tile([128, B * K], fp32, tag="PP")
                    segs = []  # (lhsT tile, wt tile, row0, rows)
                    kdec = base  # decoded channels
                    if kdec <= 128:
                        segs.append((ysb_lo, wt_lo, 0, kdec))
                    else:
                        segs.append((ysb_lo, wt_lo, 0, 128))
                        segs.append((ysb_hi, wt_hi, 0, kdec - 128))
                    for k in range(K):
                        for si, (ys, wtile, r0, rows) in enumerate(segs):
                            nc.tensor.matmul(
                                PP[:, k * B : (k + 1) * B],
                                ys[r0 : r0 + rows, k * 128 : (k + 1) * 128],
                                wtile[r0 : r0 + rows, base : base + B],
                                start=(si == 0),
                                stop=(si == len(segs) - 1),
                            )
                    # q = XMB_block - P ; layout [c*K + k] (contiguous per chan)
                    q = scratch.tile([128, B * K], fp32, tag="q")
                    nc.vector.tensor_sub(
                        q[:].rearrange("p (c k) -> p c k", k=K),
                        XMB[:].rearrange("p (k c) -> p k c", c=C)[
                            :, :, base : base + B
                        ].rearrange("p k c -> p c k"),
                        PP[:].rearrange("p (k c) -> p k c", c=B).rearrange(
                            "p k c -> p c k"
                        ),
                    )
                    qcol = lambda i: q[:, i * K : (i + 1) * K]
                else:
                    qcol = lambda i: col(XMB, i)

                # ---- in-block sequential recurrence ----
                prod = scratch.tile([128, (B - 1) * K], fp32, tag="prod")
                for i in range(B):
                    ch = base + i
                    if i > 0:
                        # d = sum_j wtri[sb,i,j] * y_j   (j < i, this block)
                        ysl = (
                            Y[:]
                            .rearrange("p (k c) -> p k c", c=C)[
                                :, :, base : base + i
                            ]
                        )
                        wrow = wtri_b[:, sb * B * B + i * B : sb * B * B + i * B + i]
                        pr = prod[:].rearrange("p (k c) -> p k c", c=B - 1)[:, :, 0:i]
                        nc.vector.tensor_tensor(
                            pr,
                            ysl,
                            wrow.unsqueeze(1).broadcast_to([128, K, i]),
                            Alu.mult,
                        )
                        d = scratch.tile([128, K], fp32, tag="d")
                        nc.vector.tensor_reduce(
                            d[:], pr, mybir.AxisListType.X, Alu.add
                        )
                        t = scratch.tile([128, K], fp32, tag="t")
                        nc.vector.tensor_sub(t[:], qcol(i), d[:])
                        t_ap = t[:]
                    else:
                        t_ap = qcol(i)

                    s = scratch.tile([128, K], fp32, tag="s")
                    nc.vector.tensor_scalar_add(s[:], t_ap, ROUND_C)
                    # m = x - t  -> mix_out column
                    nc.vector.tensor_sub(col(MIX, ch), col(X, ch), t_ap)
                    # y = (s - ROUND_C) + m
                    nc.vector.scalar_tensor_tensor(
                        col(Y, ch),
                        s[:],
                        ROUND_C,
                        col(MIX, ch),
                        op0=Alu.subtract,
                        op1=Alu.add,
                    )

                # ---- transpose finished block into channel-partition ysb ----
                if sb < NBLK - 1:
                    if base < 128:
                        dst, dr0 = ysb_lo, base
                    else:
                        dst, dr0 = ysb_hi, base - 128
                    for g in range(0, K, 4):
                        gn = min(4, K - g)
                        pt = psumt.tile([B, 512], fp32, tag="pt")
                        for t_i in range(gn):
                            k = g + t_i
                            nc.tensor.transpose(
                                pt[:, t_i * 128 : (t_i + 1) * 128],
                                Y[:, k * C + base : k * C + base + B],
                                id_t[:],
                            )
                        nc.scalar.copy(
                            dst[dr0 : dr0 + B, g * 128 : g * 128 + gn * 128],
                            pt[:, 0 : gn * 128],
                        )

            def big_out(dram, tile):
                nc.sync.dma_start(
                    dram[:].rearrange("(k p) c -> p k c", p=128),
                    tile[:].rearrange("p (k c) -> p k c", c=C),
                )

            big_out(yt, Y)
            big_out(mot, MIX)

    nc.compile()
    return nc


def get_nc(n_chunks=NPIX // 128):
    if n_chunks not in _CACHE:
        _CACHE[n_chunks] = _build(n_chunks)
    return _CACHE[n_chunks]


def make_core_inputs(x, mix, W, b):
    """Host-side layout prep. Returns list of per-core input dicts."""
    Wm = (W * np.tril(np.ones((C - 1, C), np.float32))).astype(np.float32)
    wt = np.zeros((C, C), np.float32)
    wt[:, 1:] = Wm.T  # wt[c, i] = Wm[i-1, c]
    wtri = np.zeros((NBLK, B, B), np.float32)
    for sb in range(NBLK):
        for i in range(1, B):
            ch = sb * B + i
            wtri[sb, i, :i] = Wm[ch - 1, sb * B : sb * B + i]
    wtri = wtri.reshape(1, -1)
    bpad = np.zeros((C,), np.float32)
    bpad[1:] = b
    ident = np.eye(128, dtype=np.float32)

    in_maps = []
    for n in range(N):
        xtn = np.ascontiguousarray(x[n].reshape(C, NPIX).T)
        mixn = np.ascontiguousarray(
            (mix[n] + bpad[:, None, None]).reshape(C, NPIX).T
        )
        in_maps.append(
            {
                "xt": xtn,
                "mixt": mixn,
                "wt": wt,
                "wtri": wtri,
                "ident": ident,
            }
        )
    return in_maps


def kernel(x, mix, W, b):
    from concourse.bass_utils import run_bass_kernel_spmd

    x = np.asarray(x, np.float32)
    mix = np.asarray(mix, np.float32)
    W = np.asarray(W, np.float32)
    b = np.asarray(b, np.float32)

    nc = get_nc()
    in_maps = make_core_inputs(x, mix, W, b)
    res = run_bass_kernel_spmd(nc, in_maps, list(range(N)))

    ybar = np.empty((N, C, H, Wd), np.float32)
    mix_out = np.empty((N, C, H, Wd), np.float32)
    for n in range(N):
        ybar[n] = res.results[n]["yt"].T.reshape(C, H, Wd)
        mix_out[n] = res.results[n]["mot"].T.reshape(C, H, Wd)
    mix_out[:, 0] = mix[:, 0]  # reference passes mix ch0 through exactly
    return ybar, mix_out


# revision 15
# speedup vs baseline: 1.3325x; 1.3325x over previous
"""Trainium2 Bass kernel for ChannelDepsModule (sequential channel recurrence).

Math (per pixel, fp32):
    m_0 = mix_0 ; ybar_0 = round(x_0 - m_0) + m_0
    for i in 1..191:
        m_i = sum_{c<i} Wm[i-1,c] * ybar_c + b[i-1] + mix_i
        ybar_i = round(x_i - m_i) + m_i
    outputs: ybar, mix_out (= m)

Device strategy (per core, one batch image, 4096 pixels):
  - pixels on SBUF partitions ([128] x 32 chunks), channels on the free dim
  - channels in 6 blocks of 32:
      * cross-block mix contributions P via TensorE matmuls
        (stationary ybar in channel-partition layout x Wm^T slice), with
        pixel-partition PSUM output; q = x - mix - b - P is written into the
        block's ybar columns ahead of time
      * in-block recurrence: one fused DVE scan per channel computes
        t_i = q_i - sum_j w_ij y_j directly (weights negated, +1 planted on
        the diagonal so the prefilled q column enters the dot), then one
        fused DVE op assembles y_i = round(t_i) + (x_i - t_i) using the
        +-1.5*2^23 magic constant (IEEE RNE == jnp.round)
      * mix_out column m_i = x_i - t_i is produced on the GpSimd engine,
        off the critical path
      * finished ybar columns are TensorE-transposed (two half-block waves)
        into channel-partition tiles for later blocks' matmuls
  - b is folded into mix on the host; mix_out channel 0 restored on host
"""

import sys

import numpy as np

if "/opt/trn_rl_repo" not in sys.path:
    sys.path.insert(0, "/opt/trn_rl_repo")

N, C, H, Wd = 8, 192, 64, 64
NPIX = H * Wd          # 4096 pixels per core
B = 32                 # channel block size
NBLK = C // B          # 6
ROUND_C = 1.5 * 2.0**23  # fp32 add of this rounds to nearest-even integer

_CACHE = {}
_DVE_OPS = {}


def _register_dve_ops():
    """Define + register the two fused DVE ops (idempotent)."""
    if _DVE_OPS:
        return _DVE_OPS
    import concourse.dve_ops as dops
    import concourse.dve_spec as ds
    from concourse.dve_spec import AluOp, Spec, Src0, Src1
    from concourse.dve_ops import CUSTOM_DVE_SPECS, OPS, DveOp
    from concourse.dve_uop import DveOpSpec

    # The stock segmented-scan machinery only implements the page-counter
    # mode; add the documented per-page *reset* behavior for scans marked
    # with `_page_reset`: at each SUB_DIM_DONE the STEP state computes
    # d <- op(init, expr) instead of op(CURR, expr).
    if not getattr(ds, "_page_reset_patched", False):
        _orig = ds._scan_overrides

        def _patched(scans, node_stage):
            seed, step = _orig(scans, node_stage)
            for sc in scans:
                if getattr(sc, "_page_reset", False):
                    d = node_stage[sc]
                    step[d] = ds._Stage(sc.op, ds._scan_init(sc), sc.expr)
            return seed, step

        ds._scan_overrides = _patched
        ds._page_reset_patched = True

    def _chaindot_ref(in0, in1, s0, s1, imm2):
        p = in0.shape[0]
        inner = in0.shape[-1]
        a = in0.reshape(p, -1, inner).astype(np.float32)
        bb = in1.reshape(p, -1, inner).astype(np.float32)
        return np.cumsum(a * bb, axis=-1, dtype=np.float32).reshape(in0.shape)

    sc = ds.scan(AluOp.ADD, Src0 * Src1)
    object.__setattr__(sc, "_page_reset", True)
    spec_cd = Spec(body=sc, reference=_chaindot_ref)

    def _quanty_ref(in0, in1, s0, s1, imm2):
        c = np.float32(s0)
        t = in0.astype(np.float32)
        return ((t + c) - c) + (in1.astype(np.float32) - t)

    spec_qy = Spec(
        body=((Src0 + ds.C0) - ds.C0) + (Src1 - Src0), reference=_quanty_ref
    )

    def _mk(name, spec, subdim):
        if any(o.name == name for o in OPS):
            op = next(o for o in OPS if o.name == name)
        else:
            shas = {}
            for ver in ("v3", "v4"):
                shas[ver] = DveOpSpec(
                    name=name, uops=ds.lower(spec, ver=ver)
                ).sha(ver)
            op = DveOp(name, spec, subdim=subdim, uops_sha=shas)
            OPS.append(op)
            CUSTOM_DVE_SPECS[name] = spec
            dops._SUB_OPCODE_FOR_NAME[name] = dops._CUSTOM_DVE_ROW_BASE + len(OPS) - 1
        return op

    _DVE_OPS["chaindot"] = _mk("CHAINDOT_SEQ_ANT", spec_cd, subdim=True)
    _DVE_OPS["quanty"] = _mk("QUANTY_ANT", spec_qy, subdim=False)
    return _DVE_OPS


def _build(n_chunks):
    """Build + compile the per-core Bass module. n_chunks pixel chunks of 128."""
    import concourse.bacc as bacc
    import concourse.mybir as mybir
    from concourse.tile import TileContext

    ops = _register_dve_ops()
    npix = n_chunks * 128
    fp32 = mybir.dt.float32

    nc = bacc.Bacc(None, target_bir_lowering=False)

    xt = nc.dram_tensor("xt", [npix, C], fp32, kind="ExternalInput")
    mixt = nc.dram_tensor("mixt", [npix, C], fp32, kind="ExternalInput")
    wt = nc.dram_tensor("wt", [C, C], fp32, kind="ExternalInput")
    wtri = nc.dram_tensor("wtri", [1, NBLK * B * B], fp32, kind="ExternalInput")
    ident = nc.dram_tensor("ident", [128, 128], fp32, kind="ExternalInput")
    yt = nc.dram_tensor("yt", [npix, C], fp32, kind="ExternalOutput")
    mot = nc.dram_tensor("mot", [npix, C], fp32, kind="ExternalOutput")

    K = n_chunks  # pixel chunks
    HB = B // 2   # transpose wave width

    with TileContext(nc) as tc:
        with (
            tc.tile_pool(name="big", bufs=1) as big,
            tc.tile_pool(name="small", bufs=1) as small,
            tc.tile_pool(name="scr", bufs=3) as scr,
            tc.tile_pool(name="qp", bufs=2) as qp,
            tc.tile_pool(name="psum_e", bufs=2, space="PSUM") as psum_e,
            tc.tile_pool(name="psum_f", bufs=1, space="PSUM") as psum_f,
            tc.tile_pool(name="psumt", bufs=2, space="PSUM") as psumt,
        ):
            # pixel-partition tiles, free layout = k*192 + c
            X = big.tile([128, K * C], fp32, tag="X")
            MIX = big.tile([128, K * C], fp32, tag="MIX")  # becomes mix_out
            XMB = big.tile([128, K * C], fp32, tag="XMB")
            Y = big.tile([128, K * C], fp32, tag="Y")
            # channel-partition decoded ybar: chans 0-127 / 128-159
            ysb_lo = big.tile([128, npix], fp32, tag="ysb_lo")
            ysb_hi = big.tile([32, npix], fp32, tag="ysb_hi")

            wt_lo = small.tile([128, C], fp32, tag="wt_lo")
            wt_hi = small.tile([64, C], fp32, tag="wt_hi")
            wtri_t = small.tile([1, NBLK * B * B], fp32, tag="wtri")
            wtri_b = small.tile([128, NBLK * B * B], fp32, tag="wtri_b")
            id_t = small.tile([128, 128], fp32, tag="ident")

            def big_in(tile, dram):
                nc.sync.dma_start(
                    tile[:].rearrange("p (k c) -> p k c", c=C),
                    dram[:].rearrange("(k p) c -> p k c", p=128),
                )

            big_in(X, xt)
            big_in(MIX, mixt)
            nc.sync.dma_start(wt_lo[:], wt[0:128, :])
            nc.sync.dma_start(wt_hi[:], wt[128:C, :])
            nc.sync.dma_start(wtri_t[:], wtri[:])
            nc.sync.dma_start(id_t[:], ident[:])
            nc.gpsimd.partition_broadcast(wtri_b[:], wtri_t[:])

            # XMB = X - (MIX + b)  (b folded into MIX on host)
            nc.vector.tensor_sub(XMB[:], X[:], MIX[:])

            def col(tile, ch):  # strided [128, K] view of channel ch
                return tile[:].rearrange("p (k c) -> p k c", c=C)[:, :, ch]

            def ycols(sb, j0, j1):  # [128, K, j1-j0] view of block sb's cols
                return (
                    Y[:]
                    .rearrange("p (k c) -> p k c", c=C)[
                        :, :, sb * B + j0 : sb * B + j1
                    ]
                )

            def xmb_slice(sb):
                return XMB[:].rearrange("p (k c) -> p k c", c=C)[
                    :, :, sb * B : (sb + 1) * B
                ]

            def p_early(sb, pp):
                """PP_e for block sb: chans [0, 32(sb-1)) — one matmul/chunk."""
                kdec = (sb - 1) * B
                for k in range(K):
                    nc.tensor.matmul(
                        pp[:, k * B : (k + 1) * B],
                        ysb_lo[0:kdec, k * 128 : (k + 1) * 128],
                        wt_lo[0:kdec, sb * B : (sb + 1) * B],
                    )

            def q_early(sb, qtmp, pp):
                """qtmp = XMB - PP_e (runs during the previous block's steps)."""
                nc.vector.tensor_sub(
                    qtmp[:].rearrange("p (k c) -> p k c", c=B),
                    xmb_slice(sb),
                    pp[:].rearrange("p (k c) -> p k c", c=B),
                )

            def p_final(sb, pp):
                """PP_f for block sb: block sb-1's 32 chans — one matmul/chunk."""
                r0 = (sb - 1) * B
                ys, wtile, rr = (
                    (ysb_lo, wt_lo, r0) if r0 < 128 else (ysb_hi, wt_hi, r0 - 128)
                )
                tp = (rr, 0) if rr not in (0, 32, 64) else None
                for k in range(K):
                    nc.tensor.matmul(
                        pp[:, k * B : (k + 1) * B],
                        ys[rr : rr + B, k * 128 : (k + 1) * 128],
                        wtile[rr : rr + B, sb * B : (sb + 1) * B],
                        tile_position=tp,
                    )

            def q_fin(sb, qtmp, pp_f):
                """Block sb's Y cols = (qtmp or XMB) - PP_f."""
                src = (
                    qtmp[:].rearrange("p (k c) -> p k c", c=B)
                    if qtmp is not None
                    else xmb_slice(sb)
                )
                nc.vector.tensor_sub(
                    ycols(sb, 0, B),
                    src,
                    pp_f[:].rearrange("p (k c) -> p k c", c=B),
                )

            def transpose_block(sb):
                """Transpose Y cols of block sb into ysb (chan-part)."""
                base = sb * B
                if base < 128:
                    dst, dr0 = ysb_lo, base
                else:
                    dst, dr0 = ysb_hi, base - 128
                for g in range(0, K, 4):
                    gn = min(4, K - g)
                    pt = psumt.tile([B, 512], fp32, tag="pt")
                    for t_i in range(gn):
                        k = g + t_i
                        nc.tensor.transpose(
                            pt[:, t_i * 128 : (t_i + 1) * 128],
                            Y[:, k * C + base : k * C + base + B],
                            id_t[:],
                        )
                    nc.scalar.copy(
                        dst[dr0 : dr0 + B, g * 128 : g * 128 + gn * 128],
                        pt[:, 0 : gn * 128],
                    )

            def steps(sb):
                base = sb * B
                for i in range(B):
                    ch = base + i
                    if i > 0:
                        prod = scr.tile([128, B * K], fp32, tag="prod")
                        pr = prod[:].rearrange("p (k c) -> p k c", c=B)[
                            :, :, 0 : i + 1
                        ]
                        woff = sb * B * B + i * B
                        wrow = (
                            wtri_b[:, woff : woff + i + 1]
                            .unsqueeze(1)
                            .broadcast_to([128, K, i + 1])
                        )
                        nc.vector._custom_dve(
                            ops["chaindot"], out=pr, in0=ycols(sb, 0, i + 1),
                            in1=wrow,
                        )
                        t_ap = prod[:].rearrange("p (k c) -> p k c", c=B)[:, :, i]
                    else:
                        t_ap = col(Y, ch)
                    # mix_out column (off critical path, on GpSimd)
                    nc.gpsimd.tensor_sub(col(MIX, ch), col(X, ch), t_ap)
                    # y = round(t) + (x - t)
                    nc.vector._custom_dve(
                        ops["quanty"], out=col(Y, ch), in0=t_ap,
                        in1=col(X, ch), s0=ROUND_C,
                    )

            # ---------------- schedule ----------------
            qtmp_cur = pp_f_cur = None
            for sb in range(NBLK):
                if sb == 0:
                    nc.vector.tensor_copy(ycols(sb, 0, B), xmb_slice(sb))
                else:
                    q_fin(sb, qtmp_cur, pp_f_cur)
                if sb + 1 < NBLK and sb >= 1:
                    # next block's early P + q (overlap this block's steps)
                    pp_e = psum_e.tile([128, B * K], fp32, tag="pp_e")
                    p_early(sb + 1, pp_e)
                    qtmp = qp.tile([128, B * K], fp32, tag="qtmp")
                    q_early(sb + 1, qtmp, pp_e)
                else:
                    qtmp = None
                steps(sb)
                if sb + 1 < NBLK:
                    transpose_block(sb)
                    pp_f = psum_f.tile([128, B * K], fp32, tag="pp_f")
                    p_final(sb + 1, pp_f)
                else:
                    pp_f = None
                qtmp_cur, pp_f_cur = qtmp, pp_f

            def big_out(dram, tile):
                nc.sync.dma_start(
                    dram[:].rearrange("(k p) c -> p k c", p=128),
                    tile[:].rearrange("p (k c) -> p k c", c=C),
                )

            big_out(yt, Y)
            big_out(mot, MIX)

    nc.compile()
    return nc


def get_nc(n_chunks=NPIX // 128):
    if n_chunks not in _CACHE:
        _CACHE[n_chunks] = _build(n_chunks)
    return _CACHE[n_chunks]


def make_core_inputs(x, mix, W, b):
    """Host-side layout prep. Returns list of per-core input dicts."""
    Wm = (W * np.tril(np.ones((C - 1, C), np.float32))).astype(np.float32)
    wt = np.zeros((C, C), np.float32)
    wt[:, 1:] = Wm.T  # wt[c, i] = Wm[i-1, c]
    # in-block triangle, negated, with +1 on the diagonal: the scan over
    # [y_0..y_{i-1}, q_i] then yields t_i = q_i - sum_j w_ij y_j directly
    wtri = np.zeros((NBLK, B, B), np.float32)
    for sb in range(NBLK):
        for i in range(1, B):
            ch = sb * B + i
            wtri[sb, i, :i] = -Wm[ch - 1, sb * B : sb * B + i]
            wtri[sb, i, i] = 1.0
    wtri = wtri.reshape(1, -1)
    bpad = np.zeros((C,), np.float32)
    bpad[1:] = b
    ident = np.eye(128, dtype=np.float32)

    in_maps = []
    for n in range(N):
        xtn = np.ascontiguousarray(x[n].reshape(C, NPIX).T)
        mixn = np.ascontiguousarray(
            (mix[n] + bpad[:, None, None]).reshape(C, NPIX).T
        )
        in_maps.append(
            {"xt": xtn, "mixt": mixn, "wt": wt, "wtri": wtri, "ident": ident}
        )
    return in_maps


def kernel(x, mix, W, b):
    from concourse.bass_utils import run_bass_kernel_spmd

    x = np.asarray(x, np.float32)
    mix = np.asarray(mix, np.float32)
    W = np.asarray(W, np.float32)
    b = np.asarray(b, np.float32)

    nc = get_nc()
    in_maps = make_core_inputs(x, mix, W, b)
    res = run_bass_kernel_spmd(nc, in_maps, list(range(N)))

    ybar = np.empty((N, C, H, Wd), np.float32)
    mix_out = np.empty((N, C, H, Wd), np.float32)
    for n in range(N):
        ybar[n] = res.results[n]["yt"].T.reshape(C, H, Wd)
        mix_out[n] = res.results[n]["mot"].T.reshape(C, H, Wd)
    mix_out[:, 0] = mix[:, 0]  # reference passes mix ch0 through exactly
    return ybar, mix_out
